# revision 1
# baseline (speedup 1.0000x reference)
"""Self-contained Trainium2 (Bass/Tile) kernel for the nn_Encoder problem.

kernel(**inputs) takes the FULL unsharded inputs (as produced by
setup_inputs()) and returns the FULL [4, 2048, 1024] fp32 output.

Internally: 8-way data-parallel over tokens (2 NeuronCores per batch row,
1024 query-tokens each; K/V computed redundantly per pair => no collectives).
Matmuls in fp32r; attention P/V and FFN G/W2 in bf16.
"""
import os
import numpy as np

import concourse.bass as bass
import concourse.bacc as bacc
import concourse.mybir as mybir
import concourse.tile as tile

F32 = mybir.dt.float32
F32R = mybir.dt.float32r
BF16 = mybir.dt.bfloat16
AF = mybir.ActivationFunctionType
ALU = mybir.AluOpType

E = 1024
FF = 4096
B, S = 4, 2048
T = 1024      # own tokens per core
R = 2048      # row tokens (for K/V)
P = 128
NE = E // P   # 8
NFF = FF // P # 32
NT = T // P   # 8
NR = R // P   # 16
EPS = 1e-5


def build(nc):
    # ---- DRAM I/O ----
    xrT = nc.dram_tensor("xrT", [E, R], F32R, kind="ExternalInput")   # row, feature-major
    xo = nc.dram_tensor("xo", [T, E], F32, kind="ExternalInput")      # own block, token-major
    wqT = nc.dram_tensor("wqT", [E, E], F32R, kind="ExternalInput")   # (Wq g1)^T / 32
    wkT = nc.dram_tensor("wkT", [E, E], F32R, kind="ExternalInput")
    wvT = nc.dram_tensor("wvT", [E, E], F32R, kind="ExternalInput")
    woT = nc.dram_tensor("woT", [E, E], F32R, kind="ExternalInput")
    w1T = nc.dram_tensor("w1T", [E, FF], F32R, kind="ExternalInput")
    w2T = nc.dram_tensor("w2T", [FF, E], BF16, kind="ExternalInput")
    bq = nc.dram_tensor("bq", [E], F32, kind="ExternalInput")
    bk = nc.dram_tensor("bk", [E], F32, kind="ExternalInput")
    bv = nc.dram_tensor("bv", [E], F32, kind="ExternalInput")
    bo = nc.dram_tensor("bo", [E], F32, kind="ExternalInput")
    nwqs = nc.dram_tensor("nwqs", [E], F32, kind="ExternalInput")   # -colsum(wqT)
    nwks = nc.dram_tensor("nwks", [E], F32, kind="ExternalInput")
    nwvs = nc.dram_tensor("nwvs", [E], F32, kind="ExternalInput")
    b1 = nc.dram_tensor("b1", [FF], F32, kind="ExternalInput")
    b2 = nc.dram_tensor("b2", [E], F32, kind="ExternalInput")
    g3 = nc.dram_tensor("g3", [E], F32, kind="ExternalInput")
    b3 = nc.dram_tensor("b3", [E], F32, kind="ExternalInput")
    ident = nc.dram_tensor("ident", [P, P], F32R, kind="ExternalInput")
    ones_in = nc.dram_tensor("ones_in", [P, 1], F32R, kind="ExternalInput")
    y = nc.dram_tensor("y", [T, E], F32, kind="ExternalOutput")

    def bcast_ap(vec_t, n):
        a = vec_t.ap()
        return bass.AP(tensor=a.tensor, offset=a.offset, ap=[[0, P], [1, n]])

    with tile.TileContext(nc) as tc:
        consts_cm = tc.tile_pool(name="consts", bufs=1)
        consts = consts_cm.__enter__()
        dram_cm = tc.tile_pool(name="dram", bufs=1, space="DRAM")
        dram = dram_cm.__enter__()

        ident_sb = consts.tile([P, P], F32R, tag="ident")
        nc.sync.dma_start(out=ident_sb, in_=ident.ap())
        ones_sb = consts.tile([P, 1], F32R, tag="ones")
        nc.sync.dma_start(out=ones_sb, in_=ones_in.ap())
        eps_row = consts.tile([1, 1], F32, tag="eps_row")
        nc.vector.memset(eps_row, EPS)
        eps_col = consts.tile([P, 1], F32, tag="eps_col")
        nc.vector.memset(eps_col, EPS)
        bq_sb = consts.tile([P, NE], F32, tag="bq")
        nc.sync.dma_start(out=bq_sb, in_=bq.ap().rearrange("(t p) -> p t", p=P))
        bk_sb = consts.tile([P, NE], F32, tag="bk")
        nc.sync.dma_start(out=bk_sb, in_=bk.ap().rearrange("(t p) -> p t", p=P))
        nwqs_sb = consts.tile([P, NE], F32, tag="nwqs")
        nc.sync.dma_start(out=nwqs_sb, in_=nwqs.ap().rearrange("(t p) -> p t", p=P))
        nwks_sb = consts.tile([P, NE], F32, tag="nwks")
        nc.sync.dma_start(out=nwks_sb, in_=nwks.ap().rearrange("(t p) -> p t", p=P))
        b1_sb = consts.tile([P, NFF], F32, tag="b1")
        nc.sync.dma_start(out=b1_sb, in_=b1.ap().rearrange("(t p) -> p t", p=P))
        bv_b = consts.tile([P, E], F32, tag="bv_b")
        nc.sync.dma_start(out=bv_b, in_=bcast_ap(bv, E))
        nwvs_b = consts.tile([P, E], F32, tag="nwvs_b")
        nc.sync.dma_start(out=nwvs_b, in_=bcast_ap(nwvs, E))
        bo_b = consts.tile([P, E], F32, tag="bo_b")
        nc.sync.dma_start(out=bo_b, in_=bcast_ap(bo, E))
        b2_b = consts.tile([P, E], F32, tag="b2_b")
        nc.sync.dma_start(out=b2_b, in_=bcast_ap(b2, E))
        g3_b = consts.tile([P, E], F32, tag="g3_b")
        nc.sync.dma_start(out=g3_b, in_=bcast_ap(g3, E))
        b3_b = consts.tile([P, E], F32, tag="b3_b")
        nc.sync.dma_start(out=b3_b, in_=bcast_ap(b3, E))

        q_d = dram.tile([E, T], F32R, tag="q_d")
        k_d = dram.tile([E, R], F32R, tag="k_d")
        v_d = dram.tile([R, E], BF16, tag="v_d")
        pt_d = dram.tile([R, T], BF16, tag="pt_d")
        h_d = dram.tile([T, E], F32, tag="h_d")
        stat_d = dram.tile([3, R], F32, tag="stat_d")   # mean, rstd, mean*rstd rows

        def sA_qkv():
            """LN1 stats + Q/K/V directly from x with LayerNorm output-fixup."""
            with tc.tile_pool(name="sAx", bufs=1) as xpool, \
                 tc.tile_pool(name="sAkeep", bufs=1) as keepp, \
                 tc.tile_pool(name="sAbc", bufs=1) as bcp:
                xt = []
                for k in range(NE):
                    x_k = xpool.tile([P, R], F32R, tag=f"x{k}", name=f"x{k}")
                    for c in range(4):
                        cs = slice(c * 512, (c + 1) * 512)
                        nc.sync.dma_start(out=x_k[:, cs], in_=xrT.ap()[k * P:(k + 1) * P, cs])
                    xt.append(x_k)
                rstd = keepp.tile([1, R], F32, tag="rstd")
                mr = keepp.tile([1, R], F32, tag="mr")
                # --- stats scope (freed before the weight pools open) ---
                with tc.tile_pool(name="sAsq", bufs=3) as sqp, \
                     tc.tile_pool(name="sArow", bufs=1) as rowp, \
                     tc.tile_pool(name="sArps", bufs=2, space="PSUM") as rpsA:
                    srow = rowp.tile([1, R], F32, tag="srow")
                    qrow = rowp.tile([1, R], F32, tag="qrow")
                    mean = rowp.tile([1, R], F32, tag="mean")
                    for c in range(R // 512):
                        cs = slice(c * 512, (c + 1) * 512)
                        ps_s = rpsA.tile([1, 512], F32, tag="ps_s")
                        ps_q = rpsA.tile([1, 512], F32, tag="ps_q")
                        for k in range(NE):
                            sq = sqp.tile([P, 512], F32R, tag="sq")
                            nc.vector.tensor_mul(sq, xt[k][:, cs], xt[k][:, cs])
                            nc.tensor.matmul(ps_s[:], ones_sb[:], xt[k][:, cs],
                                             start=(k == 0), stop=(k == NE - 1))
                            nc.tensor.matmul(ps_q[:], ones_sb[:], sq[:],
                                             start=(k == 0), stop=(k == NE - 1))
                        nc.vector.tensor_copy(out=srow[:, cs], in_=ps_s[:])
                        nc.vector.tensor_copy(out=qrow[:, cs], in_=ps_q[:])
                    nc.vector.tensor_scalar_mul(mean, srow[:], 1.0 / E)
                    msq = rowp.tile([1, R], F32, tag="tmp", bufs=2, name="msq")
                    nc.vector.tensor_mul(msq, mean[:], mean[:])
                    var = rowp.tile([1, R], F32, tag="tmp", bufs=2, name="var")
                    nc.vector.scalar_tensor_tensor(out=var, in0=qrow[:], scalar=1.0 / E,
                                                   in1=msq[:], op0=ALU.mult, op1=ALU.subtract)
                    sd = rowp.tile([1, R], F32, tag="tmp", bufs=2, name="sd")
                    nc.scalar.activation(out=sd, in_=var[:], func=AF.Sqrt, bias=eps_row[:],
                                         scale=1.0)
                    nc.vector.reciprocal(rstd, sd[:])
                    nc.vector.tensor_mul(mr, mean[:], rstd[:])
                    nc.sync.dma_start(out=stat_d[0:1, :], in_=mean[:])
                    nc.sync.dma_start(out=stat_d[1:2, :], in_=rstd[:])
                r_b = bcp.tile([P, R], F32, tag="r_b")
                nc.gpsimd.partition_broadcast(r_b, rstd[:])
                mr_b = bcp.tile([P, R], F32, tag="mr_b")
                nc.gpsimd.partition_broadcast(mr_b, mr[:])
                m_v = bcp.tile([P, NR], F32, tag="m_v")
                nc.sync.dma_start(out=m_v, in_=stat_d[0:1, :].rearrange("a (t p) -> (a p) t", p=P))
                r_v = bcp.tile([P, NR], F32, tag="r_v")
                nc.sync.dma_start(out=r_v, in_=stat_d[1:2, :].rearrange("a (t p) -> (a p) t", p=P))
                wvwork_cm = tc.tile_pool(name="sAwv", bufs=1)
                wvp = wvwork_cm.__enter__()
                wsp_cm = tc.tile_pool(name="sAw", bufs=3)
                wsp = wsp_cm.__enter__()
                outp_cm = tc.tile_pool(name="sAout", bufs=2)
                outp = outp_cm.__enter__()
                fxp_cm = tc.tile_pool(name="sAfix", bufs=2)
                fxp = fxp_cm.__enter__()
                psA_cm = tc.tile_pool(name="sAps", bufs=2, space="PSUM")
                psA = psA_cm.__enter__()

                # --- V0 = x^T-stationary @ wvT, fixup to token-major V (bf16) ---
                wv_t = {}
                for k in range(NE):
                    for c in range(2):
                        w = wvp.tile([P, 512], F32R, tag=f"wv{k}_{c}", name=f"wv{k}_{c}")
                        nc.sync.dma_start(out=w, in_=wvT.ap()[k * P:(k + 1) * P,
                                                              c * 512:(c + 1) * 512])
                        wv_t[(k, c)] = w
                for rm in range(NR):
                    psv = psA.tile([P, E], F32, tag="psv")
                    for k in range(NE):
                        for c in range(2):
                            nc.tensor.matmul(psv[:, c * 512:(c + 1) * 512],
                                             xt[k][:, rm * P:(rm + 1) * P], wv_t[(k, c)][:],
                                             start=(k == 0), stop=(k == NE - 1))
                    w0 = fxp.tile([P, E], F32, tag="vfx", name="vfx0")
                    nc.vector.scalar_tensor_tensor(out=w0, in0=nwvs_b[:], scalar=m_v[:, rm:rm + 1],
                                                   in1=psv[:], op0=ALU.mult, op1=ALU.add)
                    t1 = fxp.tile([P, E], F32, tag="vfx", name="vfx1")
                    nc.vector.tensor_scalar_mul(t1, w0[:], r_v[:, rm:rm + 1])
                    v_sb = outp.tile([P, E], BF16, tag="v_sb")
                    nc.gpsimd.tensor_add(v_sb, t1[:], bv_b[:])
                    nc.sync.dma_start(out=v_d[rm * P:(rm + 1) * P, :], in_=v_sb[:])

                # --- Q0/K0 weight-stationary, fixup feature-major ---
                def qk_block(wT_d, out_d, ncols, nws_sb, bias_sb):
                    nh = ncols // T   # 1 for Q, 2 for K
                    for mg in range(2):
                        w_g = []
                        for k in range(NE):
                            w = wsp.tile([P, 512], F32R, tag="w_s", bufs=10, name="w_s")
                            nc.sync.dma_start(out=w, in_=wT_d.ap()[k * P:(k + 1) * P,
                                                                   mg * 512:(mg + 1) * 512])
                            w_g.append(w)
                        for mi in range(4):
                            m = mg * 4 + mi
                            for half in range(nh):
                                hs = slice(half * T, (half + 1) * T)
                                psq = psA.tile([P, T], F32, tag="psqk", name="psqk")
                                for k in range(NE):
                                    for c in range(2):
                                        src = slice(half * T + c * 512, half * T + (c + 1) * 512)
                                        nc.tensor.matmul(psq[:, c * 512:(c + 1) * 512],
                                                         w_g[k][:, mi * P:(mi + 1) * P],
                                                         xt[k][:, src],
                                                         start=(k == 0), stop=(k == NE - 1))
                                t0 = fxp.tile([P, T], F32, tag="qkfx", name="qkfx0")
                                nc.vector.scalar_tensor_tensor(out=t0, in0=psq[:], scalar=1.0,
                                                               in1=r_b[:, hs], op0=ALU.mult,
                                                               op1=ALU.mult)
                                u = fxp.tile([P, T], F32, tag="qkfx", name="qkfx1")
                                nc.vector.tensor_scalar(out=u, in0=mr_b[:, hs],
                                                        scalar1=nws_sb[:, m:m + 1],
                                                        scalar2=bias_sb[:, m:m + 1],
                                                        op0=ALU.mult, op1=ALU.add)
                                o_sb = outp.tile([P, T], F32R, tag="qk_sb", name="qk_sb")
                                nc.gpsimd.tensor_add(o_sb, t0[:], u[:])
                                nc.sync.dma_start(
                                    out=out_d[m * P:(m + 1) * P, half * T:(half + 1) * T],
                                    in_=o_sb[:])
                qk_block(wqT, q_d, T, nwqs_sb, bq_sb)
                qk_block(wkT, k_d, R, nwks_sb, bk_sb)
                psA_cm.__exit__(None, None, None)
                fxp_cm.__exit__(None, None, None)
                outp_cm.__exit__(None, None, None)
                wsp_cm.__exit__(None, None, None)
                wvwork_cm.__exit__(None, None, None)

        def sB_scores():
            """S = Q^T K, exp (|S|<2 by construction, no max pass), P^T -> bf16."""
            with tc.tile_pool(name="sBq", bufs=1) as qp, \
                 tc.tile_pool(name="sBk", bufs=1) as kp, \
                 tc.tile_pool(name="sBw", bufs=1) as wkb, \
                 tc.tile_pool(name="sBsm", bufs=2) as smp, \
                 tc.tile_pool(name="sBps", bufs=1, space="PSUM") as psB, \
                 tc.tile_pool(name="sBtp", bufs=4, space="PSUM") as psBt:
                q_sb = []
                k_sb = []
                for m in range(NE):
                    qt = qp.tile([P, T], F32R, tag=f"q{m}", name=f"q{m}")
                    nc.sync.dma_start(out=qt, in_=q_d[m * P:(m + 1) * P, :])
                    q_sb.append(qt)
                    kt_ = kp.tile([P, R], F32R, tag=f"k{m}", name=f"k{m}")
                    nc.sync.dma_start(out=kt_, in_=k_d[m * P:(m + 1) * P, :])
                    k_sb.append(kt_)
                p_tiles = []
                for qm in range(NT):
                    qs = slice(qm * P, (qm + 1) * P)
                    p_tile = wkb.tile([P, R], F32R, tag=f"p{qm}", name=f"p{qm}")
                    p_tiles.append(p_tile)
                    acc = smp.tile([P, 4], F32, tag="acc")
                    ps_s = [psB.tile([P, 512], F32, tag=f"ps_s{c}", name=f"ps_s{c}")
                            for c in range(4)]
                    for k in range(NE):
                        for c in range(4):
                            nc.tensor.matmul(ps_s[c][:], q_sb[k][:, qs],
                                             k_sb[k][:, c * 512:(c + 1) * 512],
                                             start=(k == 0), stop=(k == NE - 1))
                    for c in range(4):
                        nc.scalar.activation(out=p_tile[:, c * 512:(c + 1) * 512],
                                             in_=ps_s[c][:],
                                             func=AF.Exp, accum_out=acc[:, c:c + 1])
                    s01 = smp.tile([P, 1], F32, tag="s01")
                    nc.vector.tensor_add(s01, acc[:, 0:1], acc[:, 1:2])
                    s23 = smp.tile([P, 1], F32, tag="s23")
                    nc.vector.tensor_add(s23, acc[:, 2:3], acc[:, 3:4])
                    rsum = smp.tile([P, 1], F32, tag="rsum")
                    nc.vector.tensor_add(rsum, s01[:], s23[:])
                    recip = smp.tile([P, 1], F32, tag="recip")
                    nc.vector.reciprocal(recip, rsum[:])
                    nc.vector.tensor_scalar_mul(p_tile, p_tile[:], recip[:])
                    for kt in range(NR):
                        tp = psBt.tile([P, P], F32R, tag="tp")
                        nc.tensor.transpose(tp, p_tiles[qm][:, kt * P:(kt + 1) * P], ident_sb[:])
                        ptc = smp.tile([P, P], BF16, tag="ptc", bufs=4, name="ptc")
                        nc.scalar.copy(out=ptc, in_=tp[:])
                        nc.sync.dma_start(out=pt_d[kt * P:(kt + 1) * P, qm * P:(qm + 1) * P],
                                          in_=ptc[:])

        def sC_attnout():
            """AOT = V^T P^T (bf16 in, fp32 psum), then O = AO^T WoT + bo, h = xo + O."""
            with tc.tile_pool(name="sCv", bufs=1) as vbp, \
                 tc.tile_pool(name="sCpt", bufs=1) as ptp, \
                 tc.tile_pool(name="sCao", bufs=1) as aop, \
                 tc.tile_pool(name="sCwo", bufs=1) as wop, \
                 tc.tile_pool(name="sCw", bufs=3) as wkc, \
                 tc.tile_pool(name="sCps", bufs=3, space="PSUM") as psC:
                v_back = []
                pt_sb = []
                for kt in range(NR):
                    vb = vbp.tile([P, E], BF16, tag=f"vb{kt}", name=f"vb{kt}")
                    nc.sync.dma_start(out=vb, in_=v_d[kt * P:(kt + 1) * P, :])
                    v_back.append(vb)
                    pb = ptp.tile([P, T], BF16, tag=f"pt{kt}", name=f"pt{kt}")
                    nc.sync.dma_start(out=pb, in_=pt_d[kt * P:(kt + 1) * P, :])
                    pt_sb.append(pb)
                wo_t = {}
                for k in range(NE):
                    for c in range(2):
                        w = wop.tile([P, 512], F32R, tag=f"wo{k}_{c}", name=f"wo{k}_{c}")
                        nc.sync.dma_start(out=w, in_=woT.ap()[k * P:(k + 1) * P,
                                                             c * 512:(c + 1) * 512])
                        wo_t[(k, c)] = w
                ao_sb = []
                for m in range(NE):
                    psa = psC.tile([P, T], F32, tag="psa", bufs=2)
                    for kt in range(NR):
                        for c in range(T // 512):
                            cs = slice(c * 512, (c + 1) * 512)
                            nc.tensor.matmul(psa[:, cs], v_back[kt][:, m * P:(m + 1) * P],
                                             pt_sb[kt][:, cs],
                                             start=(kt == 0), stop=(kt == NR - 1))
                    ao_m = aop.tile([P, T], F32R, tag=f"ao{m}", name=f"ao{m}")
                    nc.scalar.copy(out=ao_m, in_=psa[:])
                    ao_sb.append(ao_m)
                for tm in range(NT):
                    pso = psC.tile([P, E], F32, tag="pso", bufs=2)
                    for k in range(NE):
                        for c in range(2):
                            nc.tensor.matmul(pso[:, c * 512:(c + 1) * 512],
                                             ao_sb[k][:, tm * P:(tm + 1) * P], wo_t[(k, c)][:],
                                             start=(k == 0), stop=(k == NE - 1))
                    xo_t = wkc.tile([P, E], F32, tag="xo_t")
                    nc.sync.dma_start(out=xo_t, in_=xo.ap()[tm * P:(tm + 1) * P, :])
                    t0 = wkc.tile([P, E], F32, tag="t0")
                    nc.vector.tensor_add(t0, pso[:], bo_b[:])
                    h_t = wkc.tile([P, E], F32, tag="h_t")
                    nc.vector.tensor_add(h_t, t0[:], xo_t[:])
                    nc.sync.dma_start(out=h_d[tm * P:(tm + 1) * P, :], in_=h_t[:])

        def sD_ffn():
            """LN2 + transpose, F1 (G bf16, resident), F2 + residual + LN3."""
            with tc.tile_pool(name="sDhn", bufs=1) as hnp, \
                 tc.tile_pool(name="sDg", bufs=1) as gp, \
                 tc.tile_pool(name="sDw", bufs=3) as wkd, \
                 tc.tile_pool(name="sDt", bufs=3) as t6, \
                 tc.tile_pool(name="sDst", bufs=2) as st6:
                hnT = [hnp.tile([P, T], F32R, tag=f"hnT{k}", name=f"hnT{k}")
                       for k in range(NE)]
                psDt_cm = tc.tile_pool(name="sDtp", bufs=2, space="PSUM")
                psDt = psDt_cm.__enter__()
                for tm in range(NT):
                    h_t = t6.tile([P, E], F32, tag="h_in", bufs=2, name="h_in")
                    nc.sync.dma_start(out=h_t, in_=h_d[tm * P:(tm + 1) * P, :])
                    stats = st6.tile([P, 2, 6], F32, tag="stats")
                    hg = h_t[:].rearrange("p (g d) -> p g d", g=2)
                    for g in range(2):
                        nc.vector.bn_stats(out=stats[:, g, :], in_=hg[:, g, :])
                    mv = st6.tile([P, 2], F32, tag="mv")
                    nc.vector.bn_aggr(out=mv, in_=stats[:])
                    sd = st6.tile([P, 1], F32, tag="sd")
                    nc.scalar.activation(out=sd, in_=mv[:, 1:2], func=AF.Sqrt,
                                         bias=eps_col[:], scale=1.0)
                    rinv = st6.tile([P, 1], F32, tag="rinv")
                    nc.vector.reciprocal(rinv, sd[:])
                    hn = t6.tile([P, E], F32R, tag="hn", bufs=2, name="hn")
                    nc.vector.tensor_scalar(out=hn, in0=h_t[:], scalar1=mv[:, 0:1],
                                            scalar2=rinv[:], op0=ALU.subtract, op1=ALU.mult)
                    for et in range(NE):
                        tp = psDt.tile([P, P], F32R, tag="tp5")
                        nc.tensor.transpose(tp, hn[:, et * P:(et + 1) * P], ident_sb[:])
                        nc.scalar.copy(out=hnT[et][:, tm * P:(tm + 1) * P], in_=tp[:])
                psDt_cm.__exit__(None, None, None)
                # F1: G = relu(W1 hn + b1), bf16, full T resident
                g_t = []
                with tc.tile_pool(name="sDpsg", bufs=3, space="PSUM") as psg6:
                    for fg in range(NFF // 4):
                        w1_g = []
                        for k in range(NE):
                            w = wkd.tile([P, 512], F32R, tag="w1_s", bufs=10, name="w1_s")
                            nc.sync.dma_start(out=w, in_=w1T.ap()[k * P:(k + 1) * P,
                                                                  fg * 512:(fg + 1) * 512])
                            w1_g.append(w)
                        for fi in range(4):
                            fm = fg * 4 + fi
                            g = gp.tile([P, T], BF16, tag=f"g{fm}", name=f"g{fm}")
                            for c in range(2):
                                cs = slice(c * 512, (c + 1) * 512)
                                psg = psg6.tile([P, 512], F32, tag="psg", bufs=4)
                                for k in range(NE):
                                    nc.tensor.matmul(psg[:],
                                                     w1_g[k][:, fi * P:(fi + 1) * P],
                                                     hnT[k][:, cs],
                                                     start=(k == 0), stop=(k == NE - 1))
                                nc.scalar.activation(out=g[:, cs], in_=psg[:], func=AF.Relu,
                                                     bias=b1_sb[:, fm:fm + 1], scale=1.0)
                            g_t.append(g)
                # F2 + residual + LN3 per 512-token chunk
                for ch in range(2):
                    with tc.tile_pool(name=f"sDpsf{ch}", bufs=1, space="PSUM") as psf6:
                        psf = [psf6.tile([P, E], F32, tag=f"psf{tm}", name=f"psf{tm}")
                               for tm in range(4)]
                        for fm in range(NFF):
                            w2r = wkd.tile([P, E], BF16, tag="w2_s", bufs=6, name="w2_s")
                            nc.sync.dma_start(out=w2r, in_=w2T.ap()[fm * P:(fm + 1) * P, :])
                            for tm in range(4):
                                gtm = ch * 4 + tm
                                for c in range(2):
                                    nc.tensor.matmul(psf[tm][:, c * 512:(c + 1) * 512],
                                                     g_t[fm][:, gtm * P:(gtm + 1) * P],
                                                     w2r[:, c * 512:(c + 1) * 512],
                                                     start=(fm == 0), stop=(fm == NFF - 1))
                        for tm in range(4):
                            gtm = ch * 4 + tm
                            h_t = t6.tile([P, E], F32, tag="h_in", bufs=2, name="h_in2")
                            nc.sync.dma_start(out=h_t, in_=h_d[gtm * P:(gtm + 1) * P, :])
                            t1 = t6.tile([P, E], F32, tag="chain", name="t1")
                            nc.vector.tensor_add(t1, psf[tm][:], h_t[:])
                            op = t6.tile([P, E], F32, tag="chain", name="op")
                            nc.vector.tensor_add(op, t1[:], b2_b[:])
                            stats = st6.tile([P, 2, 6], F32, tag="stats7")
                            og = op[:].rearrange("p (g d) -> p g d", g=2)
                            for g in range(2):
                                nc.vector.bn_stats(out=stats[:, g, :], in_=og[:, g, :])
                            mv = st6.tile([P, 2], F32, tag="mv7")
                            nc.vector.bn_aggr(out=mv, in_=stats[:])
                            sd = st6.tile([P, 1], F32, tag="sd7")
                            nc.scalar.activation(out=sd, in_=mv[:, 1:2], func=AF.Sqrt,
                                                 bias=eps_col[:], scale=1.0)
                            rinv = st6.tile([P, 1], F32, tag="rinv7")
                            nc.vector.reciprocal(rinv, sd[:])
                            n = t6.tile([P, E], F32, tag="chain", name="n")
                            nc.vector.tensor_scalar(out=n, in0=op[:], scalar1=mv[:, 0:1],
                                                    scalar2=rinv[:], op0=ALU.subtract,
                                                    op1=ALU.mult)
                            yg = t6.tile([P, E], F32, tag="chain", name="yg")
                            nc.vector.tensor_mul(yg, n[:], g3_b[:])
                            yt = t6.tile([P, E], F32, tag="chain", name="yt")
                            nc.vector.tensor_add(yt, yg[:], b3_b[:])
                            nc.sync.dma_start(out=y.ap()[gtm * P:(gtm + 1) * P, :], in_=yt[:])

        stages = [sA_qkv, sB_scores, sC_attnout, sD_ffn]
        for _rep in range(int(os.environ.get("ENC_REPS", "1"))):
            for f in stages:
                f()

        consts_cm.__exit__(None, None, None)
        dram_cm.__exit__(None, None, None)


# ======================= host-side prep / assembly =========================

def prep_inputs(inputs):
    import ml_dtypes
    src = np.asarray(inputs["src_embs"], np.float32)   # [B, S, E]
    g1 = np.asarray(inputs["g1"], np.float32)
    b1ln = np.asarray(inputs["b1"], np.float32)
    g2 = np.asarray(inputs["g2"], np.float32)
    b2ln = np.asarray(inputs["b2"], np.float32)

    Wq, bq = np.asarray(inputs["Wq_w"], np.float32), np.asarray(inputs["Wq_b"], np.float32)
    Wk, bk = np.asarray(inputs["Wk_w"], np.float32), np.asarray(inputs["Wk_b"], np.float32)
    Wv, bv = np.asarray(inputs["Wv_w"], np.float32), np.asarray(inputs["Wv_b"], np.float32)
    Wo, bo = np.asarray(inputs["Wo_w"], np.float32), np.asarray(inputs["Wo_b"], np.float32)
    W1, b1f = np.asarray(inputs["W1_w"], np.float32), np.asarray(inputs["W1_b"], np.float32)
    W2, b2f = np.asarray(inputs["W2_w"], np.float32), np.asarray(inputs["W2_b"], np.float32)

    scale = 1.0 / np.sqrt(np.float32(E))
    wqT = ((Wq * g1[None, :]).T * scale).astype(np.float32)
    bq_eff = ((bq + Wq @ b1ln) * scale).astype(np.float32)
    wkT = (Wk * g1[None, :]).T.astype(np.float32)
    bk_eff = (bk + Wk @ b1ln).astype(np.float32)
    wvT = (Wv * g1[None, :]).T.astype(np.float32)
    bv_eff = (bv + Wv @ b1ln).astype(np.float32)
    woT = Wo.T.astype(np.float32)
    w1T = (W1 * g2[None, :]).T.astype(np.float32)
    b1_eff = (b1f + W1 @ b2ln).astype(np.float32)
    w2T = W2.T.astype(ml_dtypes.bfloat16)

    shared = dict(
        wqT=np.ascontiguousarray(wqT), wkT=np.ascontiguousarray(wkT),
        wvT=np.ascontiguousarray(wvT), woT=np.ascontiguousarray(woT),
        w1T=np.ascontiguousarray(w1T), w2T=np.ascontiguousarray(w2T),
        bq=bq_eff, bk=bk_eff, bv=bv_eff, bo=bo,
        nwqs=(-wqT.sum(axis=0)).astype(np.float32),
        nwks=(-wkT.sum(axis=0)).astype(np.float32),
        nwvs=(-wvT.sum(axis=0)).astype(np.float32),
        b1=b1_eff, b2=b2f,
        g3=np.asarray(inputs["g3"], np.float32), b3=np.asarray(inputs["b3"], np.float32),
        ident=np.eye(P, dtype=np.float32),
        ones_in=np.ones((P, 1), np.float32),
    )
    in_maps = []
    for c in range(8):
        b, half = c // 2, c % 2
        row = src[b]
        own = row[half * T:(half + 1) * T]
        other = row[(1 - half) * T:(2 - half) * T]
        xr = np.concatenate([own, other], axis=0)
        m = dict(shared)
        m["xrT"] = np.ascontiguousarray(xr.T)
        m["xo"] = np.ascontiguousarray(own)
        in_maps.append(m)
    return in_maps


def assemble_output(results):
    out = np.zeros((B, S, E), np.float32)
    for c in range(8):
        b, half = c // 2, c % 2
        out[b, half * T:(half + 1) * T] = results[c]["y"]
    return out


def build_nc():
    nc = bacc.Bacc("TRN2", target_bir_lowering=False, debug=False)
    build(nc)
    nc.compile()
    return nc


_CACHE = {}


def _get_nc():
    if "nc" not in _CACHE:
        _CACHE["nc"] = build_nc()
    return _CACHE["nc"]


def kernel(**inputs):
    from concourse import bass_utils
    nc = _get_nc()
    in_maps = prep_inputs(inputs)
    res = bass_utils.run_bass_kernel_spmd(nc, in_maps, core_ids=list(range(8)))
    return assemble_output(res.results)



# revision 2
# speedup vs baseline: 1.1596x; 1.1596x over previous
"""Trainium2 Bass kernel for nn_Encoder — fp8 DoubleRow datapath.

8-way data-parallel over tokens (2 cores per batch row, 1024 query tokens
each; K/V computed redundantly per pair => no collectives).

All eight big GEMMs (Q/K/V proj, scores, P@V, Wo, F1, F2) run as fp8e4m3
DoubleRow matmuls (2 MACs/cell/cycle).  Weights are pre-scaled x256 on host
(avoids fp8 subnormals), unscaled in the PSUM fixups.  LayerNorm1 is folded
into the projections via the colsum-fixup trick.  Scores are computed
directly transposed (S^T = K^T Q per tile), softmax normalization is
deferred past Wo (linear), row-sums come from a ones-stationary matmul.
"""
import os
import numpy as np

import concourse.bass as bass
import concourse.bacc as bacc
import concourse.mybir as mybir
import concourse.tile as tile

F32 = mybir.dt.float32
BF16 = mybir.dt.bfloat16
FP8 = mybir.dt.float8e4
AF = mybir.ActivationFunctionType
ALU = mybir.AluOpType
PM = mybir.MatmulPerfMode
DR = PM.DoubleRow

E = 1024
FF = 4096
B, S = 4, 2048
T = 1024      # own tokens per core
R = 2048      # row tokens (keys/values)
P = 128
NES = E // P    # 8  feature subtiles
NEP = NES // 2  # 4  feature pairs
NFS = FF // P   # 32 ff subtiles
NFP = NFS // 2  # 16 ff pairs
NKT = R // P    # 16 key-token tiles
NTM = T // P    # 8  own-token tiles
EPS = 1e-5
LN4 = float(np.log(4.0))
Q256 = 1.0 / 256.0


def build(nc):
    # ---- DRAM I/O ----
    xt8 = nc.dram_tensor("xt8", [P, NES, R], FP8, kind="ExternalInput")   # fp8 x, feature-major
    xo4 = nc.dram_tensor("xo4", [T, E], F32, kind="ExternalInput")        # own x + bo
    wq8 = nc.dram_tensor("wq8", [P, NES, E], FP8, kind="ExternalInput")
    wk8 = nc.dram_tensor("wk8", [P, NES, E], FP8, kind="ExternalInput")
    wv8 = nc.dram_tensor("wv8", [P, NES, E], FP8, kind="ExternalInput")
    wo8 = nc.dram_tensor("wo8", [P, NES, E], FP8, kind="ExternalInput")
    w18 = nc.dram_tensor("w18", [P, NES, FF], FP8, kind="ExternalInput")
    w28 = nc.dram_tensor("w28", [P, NFS, E], FP8, kind="ExternalInput")
    nwq_c = nc.dram_tensor("nwq_c", [P, NES], F32, kind="ExternalInput")  # -colsum(wq8)
    bq_c = nc.dram_tensor("bq_c", [P, NES], F32, kind="ExternalInput")
    nwk_c = nc.dram_tensor("nwk_c", [P, NES], F32, kind="ExternalInput")
    bk_c = nc.dram_tensor("bk_c", [P, NES], F32, kind="ExternalInput")
    b1_c = nc.dram_tensor("b1_c", [P, NFS], F32, kind="ExternalInput")
    nwv_r = nc.dram_tensor("nwv_r", [E], BF16, kind="ExternalInput")      # -colsum(wv8)
    bv_r = nc.dram_tensor("bv_r", [E], BF16, kind="ExternalInput")
    b2_r = nc.dram_tensor("b2_r", [E], BF16, kind="ExternalInput")
    g3_r = nc.dram_tensor("g3_r", [E], F32, kind="ExternalInput")
    b3_r = nc.dram_tensor("b3_r", [E], F32, kind="ExternalInput")
    ones8 = nc.dram_tensor("ones8", [P, NKT, 16], FP8, kind="ExternalInput")
    identb = nc.dram_tensor("identb", [P, P], BF16, kind="ExternalInput")
    y = nc.dram_tensor("y", [T, E], F32, kind="ExternalOutput")

    def bcast(row_ap, n):
        a = row_ap
        return bass.AP(tensor=a.tensor, offset=a.offset, ap=[[0, P], [1, n]])

    with tile.TileContext(nc) as tc:
        consts_cm = tc.tile_pool(name="consts", bufs=1, side="right")
        consts = consts_cm.__enter__()
        dram_cm = tc.tile_pool(name="dram", bufs=1, space="DRAM")
        dram = dram_cm.__enter__()

        ones_sb = consts.tile([P, NKT, 16], FP8, tag="ones")
        nc.sync.dma_start(out=ones_sb, in_=ones8.ap())
        identb_sb = consts.tile([P, P], BF16, tag="identb")
        nc.sync.dma_start(out=identb_sb, in_=identb.ap())
        eps_row = consts.tile([1, 1], F32, tag="eps_row")
        nc.vector.memset(eps_row, EPS)
        eps_col = consts.tile([P, 1], F32, tag="eps_col")
        nc.vector.memset(eps_col, EPS)
        mln4 = consts.tile([P, 1], F32, tag="mln4")
        nc.vector.memset(mln4, -LN4)
        nwq_sb = consts.tile([P, NES], F32, tag="nwq")
        nc.sync.dma_start(out=nwq_sb, in_=nwq_c.ap())
        bq_sb = consts.tile([P, NES], F32, tag="bq")
        nc.sync.dma_start(out=bq_sb, in_=bq_c.ap())
        nwk_sb = consts.tile([P, NES], F32, tag="nwk")
        nc.sync.dma_start(out=nwk_sb, in_=nwk_c.ap())
        bk_sb = consts.tile([P, NES], F32, tag="bk")
        nc.sync.dma_start(out=bk_sb, in_=bk_c.ap())
        b1_sb = consts.tile([P, NFS], F32, tag="b1")
        nc.sync.dma_start(out=b1_sb, in_=b1_c.ap())
        nwv_b = consts.tile([P, E], BF16, tag="nwv_b")
        nc.sync.dma_start(out=nwv_b, in_=bcast(nwv_r.ap(), E))
        bv_b = consts.tile([P, E], BF16, tag="bv_b")
        nc.sync.dma_start(out=bv_b, in_=bcast(bv_r.ap(), E))
        b2_b = consts.tile([P, E], BF16, tag="b2_b")
        nc.sync.dma_start(out=b2_b, in_=bcast(b2_r.ap(), E))
        g3_b = consts.tile([P, E], F32, tag="g3_b")
        nc.sync.dma_start(out=g3_b, in_=bcast(g3_r.ap(), E))
        b3_b = consts.tile([P, E], F32, tag="b3_b")
        nc.sync.dma_start(out=b3_b, in_=bcast(b3_r.ap(), E))

        stat_d = dram.tile([2, R], BF16, tag="stat_d")
        rec_d = dram.tile([1, T], F32, tag="rec_d")

        def rep():
            # h lives on the right stack for the whole rep
            hp_cm = tc.tile_pool(name="hp", bufs=1, side="right")
            hp = hp_cm.__enter__()
            h = hp.tile([P, NTM, E], F32, tag="h")

            # ---- left stack, attention phase (opened in reverse close order)
            at_cm = tc.tile_pool(name="at", bufs=1)
            at = at_cm.__enter__()
            wop_cm = tc.tile_pool(name="wop", bufs=1)
            wop = wop_cm.__enter__()
            vp_cm = tc.tile_pool(name="vp", bufs=1)
            vp = vp_cm.__enter__()
            kq_cm = tc.tile_pool(name="kq", bufs=1)
            kq = kq_cm.__enter__()
            xin_cm = tc.tile_pool(name="xin", bufs=1)
            xin = xin_cm.__enter__()
            wqkv_cm = tc.tile_pool(name="wqkv", bufs=2)
            wqkv = wqkv_cm.__enter__()
            stats_cm = tc.tile_pool(name="stats", bufs=1)
            stats = stats_cm.__enter__()

            P8 = at.tile([P, NKT, T], FP8, tag="P8")
            AO = at.tile([P, NES, T], FP8, tag="AO")
            wo = wop.tile([P, NES, E], FP8, tag="wo")
            for s2 in range(2):
                nc.sync.dma_start(out=wo[:, 4 * s2:4 * s2 + 4, :],
                                  in_=wo8.ap()[:, 4 * s2:4 * s2 + 4, :])
            V8 = vp.tile([P, NKT, E], FP8, tag="V8")
            K8 = kq.tile([P, NES, R], FP8, tag="K8")
            Q8 = kq.tile([P, NES, T], FP8, tag="Q8")

            xt = xin.tile([P, NES, R], FP8, tag="xt")
            for s2 in range(4):
                nc.sync.dma_start(out=xt[:, 2 * s2:2 * s2 + 2, :],
                                  in_=xt8.ap()[:, 2 * s2:2 * s2 + 2, :])

            def load_w(w_d):
                w_sb = wqkv.tile([P, NES, E], FP8, tag="w")
                for s2 in range(2):
                    nc.sync.dma_start(out=w_sb[:, 4 * s2:4 * s2 + 4, :],
                                      in_=w_d.ap()[:, 4 * s2:4 * s2 + 4, :])
                return w_sb

            # ---------- LN1 stats over the full row ----------
            srow = stats.tile([1, R], F32, tag="row", name="srow", bufs=3)
            qrow = stats.tile([1, R], F32, tag="row", name="qrow", bufs=3)
            with tc.tile_pool(name="sqp", bufs=2) as sqp, \
                 tc.tile_pool(name="psS", bufs=1, space="PSUM") as psS:
                for half in range(2):
                    ps_s = psS.tile([16, T], F32, tag="ps_s", name="ps_s")
                    ps_q = psS.tile([16, T], F32, tag="ps_q", name="ps_q")
                    for kc in range(4):
                        sl = slice(half * T + kc * 256, half * T + (kc + 1) * 256)
                        ol = slice(kc * 256, (kc + 1) * 256)
                        for j in range(NEP):
                            nc.tensor.matmul(ps_s[:, ol], ones_sb[:, 2 * j:2 * j + 2, :],
                                             xt[:, 2 * j:2 * j + 2, sl],
                                             start=(j == 0 and kc % 2 == 0),
                                             stop=(j == NEP - 1 and kc % 2 == 1),
                                             perf_mode=DR)
                        sq_c = sqp.tile([P, NES, 256], FP8, tag="sq")
                        nc.vector.tensor_mul(sq_c, xt[:, :, sl], xt[:, :, sl])
                        for j in range(NEP):
                            nc.tensor.matmul(ps_q[:, ol], ones_sb[:, 2 * j:2 * j + 2, :],
                                             sq_c[:, 2 * j:2 * j + 2, :],
                                             start=(j == 0 and kc % 2 == 0),
                                             stop=(j == NEP - 1 and kc % 2 == 1),
                                             perf_mode=DR)
                    hl = slice(half * T, (half + 1) * T)
                    nc.vector.tensor_copy(out=srow[:, hl], in_=ps_s[0:1, :])
                    nc.vector.tensor_copy(out=qrow[:, hl], in_=ps_q[0:1, :])
            mean = stats.tile([1, R], F32, tag="mean")
            nc.vector.tensor_scalar_mul(mean, srow[:], 1.0 / E)
            msq = stats.tile([1, R], F32, tag="row", name="msq", bufs=3)
            nc.vector.tensor_mul(msq, mean[:], mean[:])
            var = stats.tile([1, R], F32, tag="row", name="var", bufs=3)
            nc.vector.scalar_tensor_tensor(out=var, in0=qrow[:], scalar=1.0 / E,
                                           in1=msq[:], op0=ALU.mult, op1=ALU.subtract)
            sd = stats.tile([1, R], F32, tag="row", name="sd", bufs=3)
            nc.scalar.activation(out=sd, in_=var[:], func=AF.Sqrt, bias=eps_row[:],
                                 scale=1.0)
            rstd = stats.tile([1, R], F32, tag="row", name="rstd", bufs=3)
            nc.vector.reciprocal(rstd, sd[:])
            r256 = stats.tile([1, R], F32, tag="row", name="r256", bufs=3)
            nc.vector.tensor_scalar_mul(r256, rstd[:], Q256)
            mean_bf = stats.tile([1, R], BF16, tag="mean_bf")
            nc.vector.tensor_copy(out=mean_bf, in_=mean[:])
            r256_bf = stats.tile([1, R], BF16, tag="r256_bf")
            nc.vector.tensor_copy(out=r256_bf, in_=r256[:])
            nc.sync.dma_start(out=stat_d[0:1, :], in_=mean_bf[:])
            nc.sync.dma_start(out=stat_d[1:2, :], in_=r256_bf[:])
            m_b = stats.tile([P, R], BF16, tag="m_b")
            nc.sync.dma_start(out=m_b, in_=bcast(stat_d[0:1, :], R))
            r_b = stats.tile([P, R], BF16, tag="r_b")
            nc.sync.dma_start(out=r_b, in_=bcast(stat_d[1:2, :], R))
            m_v = stats.tile([P, NKT], BF16, tag="m_v")
            nc.sync.dma_start(out=m_v,
                              in_=stat_d[0:1, :].rearrange("a (t p) -> (a p) t", p=P))
            r_v = stats.tile([P, NKT], BF16, tag="r_v")
            nc.sync.dma_start(out=r_v,
                              in_=stat_d[1:2, :].rearrange("a (t p) -> (a p) t", p=P))
            mr_v = stats.tile([P, NKT], F32, tag="mr_v")
            nc.vector.tensor_mul(mr_v, m_v[:], r_v[:])

            # ---------- K / Q projections (feature-major out) ----------
            with tc.tile_pool(name="psKQ", bufs=2, space="PSUM") as psKQ, \
                 tc.tile_pool(name="fxKQ", bufs=2) as fxKQ:
                def kq_proj(w_sb, out_sb, ncols, nw_sb, b_sb):
                    for es in range(NES):
                        for half in range(ncols // T):
                            hl = slice(half * T, (half + 1) * T)
                            ps = psKQ.tile([P, T], F32, tag="psKQ", name="psKQ")
                            for j in range(NEP):
                                for c4 in range(4):
                                    ol = slice(c4 * 256, (c4 + 1) * 256)
                                    sl = slice(half * T + c4 * 256,
                                               half * T + (c4 + 1) * 256)
                                    nc.tensor.matmul(
                                        ps[:, ol],
                                        w_sb[:, 2 * j:2 * j + 2, es * P:(es + 1) * P],
                                        xt[:, 2 * j:2 * j + 2, sl],
                                        start=(j == 0 and c4 % 2 == 0),
                                        stop=(j == NEP - 1 and c4 % 2 == 1),
                                        perf_mode=DR)
                            t0 = fxKQ.tile([P, T], BF16, tag="t0", name="t0")
                            nc.vector.scalar_tensor_tensor(
                                out=t0, in0=m_b[:, hl], scalar=nw_sb[:, es:es + 1],
                                in1=ps[:], op0=ALU.mult, op1=ALU.add)
                            t1 = fxKQ.tile([P, T], BF16, tag="t1", name="t1")
                            nc.vector.tensor_mul(t1, t0[:], r_b[:, hl])
                            nc.gpsimd.tensor_scalar_add(
                                out=out_sb[:, es:es + 1, hl], in0=t1[:],
                                scalar1=b_sb[:, es:es + 1])
                wk = load_w(wk8)
                kq_proj(wk, K8, R, nwk_sb, bk_sb)
                wq = load_w(wq8)
                kq_proj(wq, Q8, T, nwq_sb, bq_sb)

                # ---------- V projection (token-major out) ----------
                wv = load_w(wv8)
                for kt in range(NKT):
                    ps = psKQ.tile([P, E], F32, tag="psKQ", name="psV")
                    for j in range(NEP):
                        for c4 in range(4):
                            ol = slice(c4 * 256, (c4 + 1) * 256)
                            nc.tensor.matmul(
                                ps[:, ol],
                                xt[:, 2 * j:2 * j + 2, kt * P:(kt + 1) * P],
                                wv[:, 2 * j:2 * j + 2, ol],
                                start=(j == 0 and c4 % 2 == 0),
                                stop=(j == NEP - 1 and c4 % 2 == 1), perf_mode=DR)
                    t0 = fxKQ.tile([P, E], BF16, tag="t0", name="vt0")
                    nc.vector.scalar_tensor_tensor(
                        out=t0, in0=nwv_b[:], scalar=mr_v[:, kt:kt + 1],
                        in1=bv_b[:], op0=ALU.mult, op1=ALU.add)
                    nc.vector.scalar_tensor_tensor(
                        out=V8[:, kt:kt + 1, :], in0=ps[:], scalar=r_v[:, kt:kt + 1],
                        in1=t0[:], op0=ALU.mult, op1=ALU.add)

            stats_cm.__exit__(None, None, None)
            wqkv_cm.__exit__(None, None, None)
            xin_cm.__exit__(None, None, None)

            # ---------- scores + exp ----------
            with tc.tile_pool(name="psSc", bufs=2, space="PSUM") as psSc:
                for kt in range(NKT):
                    ps = psSc.tile([P, T], F32, tag="psSc", name="psSc")
                    for j in range(NEP):
                        for c4 in range(4):
                            ol = slice(c4 * 256, (c4 + 1) * 256)
                            nc.tensor.matmul(
                                ps[:, ol],
                                K8[:, 2 * j:2 * j + 2, kt * P:(kt + 1) * P],
                                Q8[:, 2 * j:2 * j + 2, ol],
                                start=(j == 0 and c4 % 2 == 0),
                                stop=(j == NEP - 1 and c4 % 2 == 1), perf_mode=DR)
                    nc.scalar.activation(out=P8[:, kt:kt + 1, :], in_=ps[:],
                                         func=AF.Exp, bias=mln4[:], scale=1.0 / 32.0)
            kq_cm.__exit__(None, None, None)

            # ---------- softmax row-sums -> recip ----------
            with tc.tile_pool(name="psRs", bufs=1, space="PSUM") as psRs, \
                 tc.tile_pool(name="rsp", bufs=1) as rsp:
                psr = psRs.tile([16, T], F32, tag="psr")
                for j in range(NKT // 2):
                    for c4 in range(4):
                        ol = slice(c4 * 256, (c4 + 1) * 256)
                        nc.tensor.matmul(psr[:, ol], ones_sb[:, 2 * j:2 * j + 2, :],
                                         P8[:, 2 * j:2 * j + 2, ol],
                                         start=(j == 0 and c4 % 2 == 0),
                                         stop=(j == NKT // 2 - 1 and c4 % 2 == 1),
                                         perf_mode=DR)
                rec = rsp.tile([1, T], F32, tag="rec")
                nc.vector.reciprocal(rec, psr[0:1, :])
                rec2 = rsp.tile([1, T], F32, tag="rec2")
                nc.vector.tensor_scalar_mul(rec2, rec[:], Q256)
                nc.sync.dma_start(out=rec_d[:], in_=rec2[:])
            r_o = at.tile([P, NTM], F32, tag="r_o")
            nc.sync.dma_start(out=r_o,
                              in_=rec_d[0:1, :].rearrange("a (t p) -> (a p) t", p=P))

            # ---------- P @ V (feature-major out, fp8) ----------
            with tc.tile_pool(name="psPV", bufs=2, space="PSUM") as psPV:
                for es in range(NES):
                    ps = psPV.tile([P, T], F32, tag="psPV", name="psPV")
                    for j in range(NKT // 2):
                        for c4 in range(4):
                            ol = slice(c4 * 256, (c4 + 1) * 256)
                            nc.tensor.matmul(
                                ps[:, ol],
                                V8[:, 2 * j:2 * j + 2, es * P:(es + 1) * P],
                                P8[:, 2 * j:2 * j + 2, ol],
                                start=(j == 0 and c4 % 2 == 0),
                                stop=(j == NKT // 2 - 1 and c4 % 2 == 1),
                                perf_mode=DR)
                    nc.scalar.copy(out=AO[:, es:es + 1, :], in_=ps[:])
            vp_cm.__exit__(None, None, None)

            # ---------- Wo + renormalize + residual ----------
            with tc.tile_pool(name="psWo", bufs=2, space="PSUM") as psWo, \
                 tc.tile_pool(name="xop", bufs=3) as xop, \
                 tc.tile_pool(name="fxO", bufs=2) as fxO:
                for tm in range(NTM):
                    ps = psWo.tile([P, E], F32, tag="psWo", name="psWo")
                    for j in range(NEP):
                        for c4 in range(4):
                            ol = slice(c4 * 256, (c4 + 1) * 256)
                            nc.tensor.matmul(
                                ps[:, ol],
                                AO[:, 2 * j:2 * j + 2, tm * P:(tm + 1) * P],
                                wo[:, 2 * j:2 * j + 2, ol],
                                start=(j == 0 and c4 % 2 == 0),
                                stop=(j == NEP - 1 and c4 % 2 == 1), perf_mode=DR)
                    xo_t = xop.tile([P, E], F32, tag="xo_t", name="xo_t")
                    nc.sync.dma_start(out=xo_t, in_=xo4.ap()[tm * P:(tm + 1) * P, :])
                    t0 = fxO.tile([P, E], F32, tag="ot0", name="ot0")
                    nc.vector.tensor_scalar_mul(t0, ps[:], r_o[:, tm:tm + 1])
                    nc.gpsimd.tensor_add(h[:, tm:tm + 1, :], t0[:], xo_t[:])
            wop_cm.__exit__(None, None, None)
            at_cm.__exit__(None, None, None)

            # ---------- FFN phase ----------
            fp_cm = tc.tile_pool(name="fp", bufs=1)
            fp = fp_cm.__enter__()
            w2p_cm = tc.tile_pool(name="w2p", bufs=1)
            w2p = w2p_cm.__enter__()
            w1p_cm = tc.tile_pool(name="w1p", bufs=1)
            w1p = w1p_cm.__enter__()
            hnT = fp.tile([P, NES, T], FP8, tag="hnT")
            G = fp.tile([P, NFS, T], FP8, tag="G")
            w2 = w2p.tile([P, NFS, E], FP8, tag="w2")
            for s8 in range(8):
                nc.sync.dma_start(out=w2[:, 4 * s8:4 * s8 + 4, :],
                                  in_=w28.ap()[:, 4 * s8:4 * s8 + 4, :])
            w1 = w1p.tile([P, NES, FF], FP8, tag="w1")
            for s8 in range(8):
                nc.sync.dma_start(out=w1[:, s8:s8 + 1, :],
                                  in_=w18.ap()[:, s8:s8 + 1, :])

            # LN2 + transpose
            with tc.tile_pool(name="ln2", bufs=3) as ln2, \
                 tc.tile_pool(name="st2", bufs=2) as st2, \
                 tc.tile_pool(name="psT", bufs=4, space="PSUM") as psT:
                for tm in range(NTM):
                    stats2 = st2.tile([P, 2, 6], F32, tag="stats2")
                    hg = h[:, tm, :].rearrange("p (g d) -> p g d", g=2)
                    for g in range(2):
                        nc.vector.bn_stats(out=stats2[:, g, :], in_=hg[:, g, :])
                    mv = st2.tile([P, 2], F32, tag="mv")
                    nc.vector.bn_aggr(out=mv, in_=stats2[:])
                    sd2 = st2.tile([P, 1], F32, tag="sd2")
                    nc.scalar.activation(out=sd2, in_=mv[:, 1:2], func=AF.Sqrt,
                                         bias=eps_col[:], scale=1.0)
                    rinv = st2.tile([P, 1], F32, tag="rinv")
                    nc.vector.reciprocal(rinv, sd2[:])
                    hn_bf = ln2.tile([P, E], BF16, tag="hn_bf", name="hn_bf")
                    nc.vector.tensor_scalar(out=hn_bf, in0=h[:, tm, :],
                                            scalar1=mv[:, 0:1], scalar2=rinv[:],
                                            op0=ALU.subtract, op1=ALU.mult)
                    for es in range(NES):
                        pst = psT.tile([P, P], BF16, tag="pst")
                        nc.tensor.transpose(pst, hn_bf[:, es * P:(es + 1) * P],
                                            identb_sb[:])
                        nc.scalar.copy(out=hnT[:, es:es + 1, tm * P:(tm + 1) * P],
                                       in_=pst[:])

            # F1
            with tc.tile_pool(name="psF1", bufs=2, space="PSUM") as psF1:
                for fs in range(NFS):
                    ps = psF1.tile([P, T], F32, tag="psF1", name="psF1")
                    for j in range(NEP):
                        for c4 in range(4):
                            ol = slice(c4 * 256, (c4 + 1) * 256)
                            nc.tensor.matmul(
                                ps[:, ol],
                                w1[:, 2 * j:2 * j + 2, fs * P:(fs + 1) * P],
                                hnT[:, 2 * j:2 * j + 2, ol],
                                start=(j == 0 and c4 % 2 == 0),
                                stop=(j == NEP - 1 and c4 % 2 == 1), perf_mode=DR)
                    nc.scalar.activation(out=G[:, fs:fs + 1, :], in_=ps[:],
                                         func=AF.Relu, bias=b1_sb[:, fs:fs + 1],
                                         scale=Q256)
            w1p_cm.__exit__(None, None, None)

            # F2 + residual + LN3
            with tc.tile_pool(name="psF2", bufs=2, space="PSUM") as psF2, \
                 tc.tile_pool(name="fx2", bufs=1) as fx2, \
                 tc.tile_pool(name="st3", bufs=2) as st3:
                for tm in range(NTM):
                    ps = psF2.tile([P, E], F32, tag="psF2", name="psF2")
                    for j in range(NFP):
                        for c4 in range(4):
                            ol = slice(c4 * 256, (c4 + 1) * 256)
                            nc.tensor.matmul(
                                ps[:, ol],
                                G[:, 2 * j:2 * j + 2, tm * P:(tm + 1) * P],
                                w2[:, 2 * j:2 * j + 2, ol],
                                start=(j == 0 and c4 % 2 == 0),
                                stop=(j == NFP - 1 and c4 % 2 == 1), perf_mode=DR)
                    o1 = fx2.tile([P, E], F32, tag="o1", name="o1")
                    nc.vector.scalar_tensor_tensor(out=o1, in0=ps[:], scalar=Q256,
                                                   in1=b2_b[:], op0=ALU.mult,
                                                   op1=ALU.add)
                    o = fx2.tile([P, E], F32, tag="o", name="o")
                    nc.gpsimd.tensor_add(o, o1[:], h[:, tm, :])
                    stats3 = st3.tile([P, 2, 6], F32, tag="stats3")
                    og = o[:].rearrange("p (g d) -> p g d", g=2)
                    for g in range(2):
                        nc.vector.bn_stats(out=stats3[:, g, :], in_=og[:, g, :])
                    mv3 = st3.tile([P, 2], F32, tag="mv3")
                    nc.vector.bn_aggr(out=mv3, in_=stats3[:])
                    sd3 = st3.tile([P, 1], F32, tag="sd3")
                    nc.scalar.activation(out=sd3, in_=mv3[:, 1:2], func=AF.Sqrt,
                                         bias=eps_col[:], scale=1.0)
                    rinv3 = st3.tile([P, 1], F32, tag="rinv3")
                    nc.vector.reciprocal(rinv3, sd3[:])
                    n = fx2.tile([P, E], F32, tag="n", name="n")
                    nc.vector.tensor_scalar(out=n, in0=o[:], scalar1=mv3[:, 0:1],
                                            scalar2=rinv3[:], op0=ALU.subtract,
                                            op1=ALU.mult)
                    yg = fx2.tile([P, E], F32, tag="yg", name="yg")
                    nc.gpsimd.tensor_mul(yg, n[:], g3_b[:])
                    yt = fx2.tile([P, E], F32, tag="yt", name="yt")
                    nc.vector.tensor_add(yt, yg[:], b3_b[:])
                    nc.sync.dma_start(out=y.ap()[tm * P:(tm + 1) * P, :], in_=yt[:])
            w2p_cm.__exit__(None, None, None)
            fp_cm.__exit__(None, None, None)
            hp_cm.__exit__(None, None, None)

        for _ in range(int(os.environ.get("ENC_REPS", "1"))):
            rep()

        dram_cm.__exit__(None, None, None)
        consts_cm.__exit__(None, None, None)


# ======================= host-side prep / assembly =========================

def prep_inputs(inputs):
    import ml_dtypes
    FP8NP = ml_dtypes.float8_e4m3
    src = np.asarray(inputs["src_embs"], np.float32)   # [B, S, E]
    g1 = np.asarray(inputs["g1"], np.float32)
    b1ln = np.asarray(inputs["b1"], np.float32)
    g2 = np.asarray(inputs["g2"], np.float32)
    b2ln = np.asarray(inputs["b2"], np.float32)

    Wq, bq = np.asarray(inputs["Wq_w"], np.float32), np.asarray(inputs["Wq_b"], np.float32)
    Wk, bk = np.asarray(inputs["Wk_w"], np.float32), np.asarray(inputs["Wk_b"], np.float32)
    Wv, bv = np.asarray(inputs["Wv_w"], np.float32), np.asarray(inputs["Wv_b"], np.float32)
    Wo, bo = np.asarray(inputs["Wo_w"], np.float32), np.asarray(inputs["Wo_b"], np.float32)
    W1, b1f = np.asarray(inputs["W1_w"], np.float32), np.asarray(inputs["W1_b"], np.float32)
    W2, b2f = np.asarray(inputs["W2_w"], np.float32), np.asarray(inputs["W2_b"], np.float32)

    def to8(mat, nsub):
        # [K, N] -> [P, nsub, N] fp8, K = sub*128 + p
        Kd, Nd = mat.shape
        assert Kd == nsub * P
        return np.ascontiguousarray(
            (mat * 256.0).reshape(nsub, P, Nd).transpose(1, 0, 2)).astype(FP8NP)

    wq8 = to8((Wq * g1[None, :]).T, NES)
    wk8 = to8((Wk * g1[None, :]).T, NES)
    wv8 = to8((Wv * g1[None, :]).T, NES)
    wo8 = to8(Wo.T, NES)
    w18 = to8((W1 * g2[None, :]).T, NES)
    w28 = to8(W2.T, NFS)

    def colsum(w8):
        # -sum over contraction of the QUANTIZED values, [N] f32
        return -w8.astype(np.float32).sum(axis=(0, 1))

    def col(vec, nsub):
        # [N] -> [P, nsub], N = sub*128 + p
        return np.ascontiguousarray(vec.reshape(nsub, P).T).astype(np.float32)

    bq_eff = (bq + Wq @ b1ln).astype(np.float32)
    bk_eff = (bk + Wk @ b1ln).astype(np.float32)
    bv_eff = (bv + Wv @ b1ln).astype(np.float32)
    b1_eff = (b1f + W1 @ b2ln).astype(np.float32)

    shared = dict(
        wq8=wq8, wk8=wk8, wv8=wv8, wo8=wo8, w18=w18, w28=w28,
        nwq_c=col(colsum(wq8), NES), bq_c=col(bq_eff, NES),
        nwk_c=col(colsum(wk8), NES), bk_c=col(bk_eff, NES),
        b1_c=col(b1_eff, NFS),
        nwv_r=colsum(wv8).astype(ml_dtypes.bfloat16),
        bv_r=bv_eff.astype(ml_dtypes.bfloat16),
        b2_r=b2f.astype(ml_dtypes.bfloat16),
        g3_r=np.asarray(inputs["g3"], np.float32),
        b3_r=np.asarray(inputs["b3"], np.float32),
        ones8=np.ones((P, NKT, 16), FP8NP),
        identb=np.eye(P).astype(ml_dtypes.bfloat16),
    )
    in_maps = []
    for c in range(8):
        b, half = c // 2, c % 2
        row = src[b]
        own = row[half * T:(half + 1) * T]
        other = row[(1 - half) * T:(2 - half) * T]
        xr = np.concatenate([own, other], axis=0)          # [R, E], own first
        xt = xr.T.reshape(NES, P, R).transpose(1, 0, 2)    # [P, NES, R]
        m = dict(shared)
        m["xt8"] = np.ascontiguousarray(xt).astype(FP8NP)
        m["xo4"] = np.ascontiguousarray(own + bo[None, :]).astype(np.float32)
        in_maps.append(m)
    return in_maps


def assemble_output(results):
    out = np.zeros((B, S, E), np.float32)
    for c in range(8):
        b, half = c // 2, c % 2
        out[b, half * T:(half + 1) * T] = results[c]["y"]
    return out


def build_nc():
    nc = bacc.Bacc("TRN2", target_bir_lowering=False, debug=False)
    build(nc)
    nc.compile()
    return nc


_CACHE = {}


def _get_nc():
    if "nc" not in _CACHE:
        _CACHE["nc"] = build_nc()
    return _CACHE["nc"]


def kernel(**inputs):
    from concourse import bass_utils
    nc = _get_nc()
    in_maps = prep_inputs(inputs)
    res = bass_utils.run_bass_kernel_spmd(nc, in_maps, core_ids=list(range(8)))
    return assemble_output(res.results)
